# revision 33
# baseline (speedup 1.0000x reference)
"""Trainium2 Bass kernel: GQA attention block (nn_Attention_66142496358763).

Full module: x -> (wq,wk,wv) projections -> RoPE(q,k) -> softmax(q k^T/sqrt(d)) v
(GQA: 32 q heads, 8 kv heads) -> wo projection.

Sharding (tensor-parallel over heads, 8 cores):
  core c: q heads [4c, 4c+4), kv head c, wq/wk/wv column shards, wo row shard
  -> each core emits a partial [S, DIM] output; host sums the 8 partials.

All TensorE math in fp16 (full-rate on trn2), fp32 PSUM accumulation,
softmax exp in fp32 on ScalarE. Softmax is computed without the max
subtraction (scores are O(10) here; a -4 bias inside exp keeps the fp16
P-matrix in range) and the denominator comes for free from a ones-column
appended to V inside the PV matmul. A/V transposes ride the DMA xbar.
"""

import numpy as np

S = 2048
DIM = 4096
HD = 128
NCORES = 8
HPC = 4          # q heads per core
QB = 512         # q block (seq block) size
NQB = S // QB    # 4
DKT = DIM // 128  # 32 contraction tiles for projections
KT = S // 128    # 16 key tiles for attention
NPAIR = KT // 2  # 8 score/exp pairs per (head, qblock)
SCALE = float(HD) ** -0.5
EXP_BIAS = -4.0

_CACHE = {}


def _build_nc():
    import concourse.bass as bass
    import concourse.tile as tile
    from concourse import bacc, mybir

    fp16 = mybir.dt.float16
    f32 = mybir.dt.float32
    AF = mybir.ActivationFunctionType

    nc = bacc.Bacc("TRN2", target_bir_lowering=False, debug=False)

    xt_d = nc.dram_tensor("xt", [DIM, S], fp16, kind="ExternalInput").ap()
    wq_d = nc.dram_tensor("wq", [DIM, HPC * HD], fp16, kind="ExternalInput").ap()
    wkv_d = nc.dram_tensor("wkv", [DIM, 2 * HD], fp16, kind="ExternalInput").ap()
    wo_d = nc.dram_tensor("wo", [HPC * HD, DIM], fp16, kind="ExternalInput").ap()
    rc_d = nc.dram_tensor("ropec", [HD, S], fp16, kind="ExternalInput").ap()
    rs_d = nc.dram_tensor("ropes", [HD, S], fp16, kind="ExternalInput").ap()
    sw_d = nc.dram_tensor("pswap", [HD, HD], fp16, kind="ExternalInput").ap()
    out_d = nc.dram_tensor("out", [S, DIM], fp16, kind="ExternalOutput").ap()

    xt_r = xt_d.rearrange("(kt p) s -> p kt s", p=128)
    wq_r = wq_d.rearrange("(kt p) n -> p kt n", p=128)
    wkv_r = wkv_d.rearrange("(kt p) n -> p kt n", p=128)
    wo_r = wo_d.rearrange("(h p) n -> p h n", p=128)
    out_r = out_d.rearrange("(st p) n -> st p n", p=128)

    with tile.TileContext(nc) as tc:
        with (
            tc.tile_pool(name="const", bufs=1) as const,
            tc.tile_pool(name="xtp", bufs=5) as xtp,
            tc.tile_pool(name="persist", bufs=1) as persist,
            tc.tile_pool(name="tmp", bufs=5) as tmp,
            tc.tile_pool(name="t12", bufs=4) as t12,
            tc.tile_pool(name="pt", bufs=4) as ptp,
            tc.tile_pool(name="asb", bufs=6) as asbp,
            tc.tile_pool(name="small", bufs=8) as small,
            tc.tile_pool(name="outp", bufs=2) as outp,
            tc.tile_pool(name="ps_big", bufs=2, space="PSUM") as ps_big,
            tc.tile_pool(name="ps_acc", bufs=4, space="PSUM") as ps_acc,
        ):
            # ---- inputs split across the two HWDGE queues:
            # sync: wk + the xt chunk stream; scalar: rope consts, wv, wq, wo
            xt_tiles = {0: []}
            for j in range(4):
                t = xtp.tile([128, 8, QB], fp16, tag="xt", name=f"xt_0_{j}")
                nc.sync.dma_start(t[:], xt_r[:, 8 * j:8 * (j + 1), 0:QB])
                xt_tiles[0].append(t)
            wkv_c = []
            wq_c = []
            for j in range(4):
                w = const.tile([128, 8, 2 * HD], fp16, tag=f"c_wkv{j}", name=f"wkv_c{j}")
                nc.scalar.dma_start(w[:], wkv_r[:, 8 * j:8 * (j + 1), :])
                wkv_c.append(w)
                q = const.tile([128, 8, 512], fp16, tag=f"c_wq{j}", name=f"wq_c{j}")
                nc.scalar.dma_start(q[:], wq_r[:, 8 * j:8 * (j + 1), :])
                wq_c.append(q)
            rc_sb = const.tile([HD, S], fp16, tag="c_rc")
            nc.scalar.dma_start(rc_sb[:], rc_d[:])
            rs_sb = const.tile([HD, S], fp16, tag="c_rs")
            nc.scalar.dma_start(rs_sb[:], rs_d[:])
            sw_sb = const.tile([HD, HD], fp16, tag="c_sw")
            nc.scalar.dma_start(sw_sb[:], sw_d[:])
            ebias_sb = const.tile([128, 1], f32, tag="c_eb")
            nc.gpsimd.memset(ebias_sb[:], EXP_BIAS)
            wo_sb = const.tile([128, HPC, DIM], fp16, tag="c_wo")

            # PE warm-up: ~9us of dummy matmuls with no input deps, so the HAM
            # clock-gate opens while the first DMAs are still streaming in
            warm_sb = const.tile([128, 128], fp16, tag="c_warm")
            nc.gpsimd.memset(warm_sb[:], 0.0)
            warm_ps = ps_big.tile([128, 1024], f32, tag="ps_big", name="warm_ps")
            for _ in range(80):
                nc.tensor.matmul(
                    warm_ps[:, 0:128], warm_sb[:], warm_sb[:], start=True, stop=True
                )

            # persistent activations
            qt_sb = persist.tile([128, HPC, S], fp16, tag="p_qt")   # rope'd Q^T per head
            kt_sb = persist.tile([128, S], fp16, tag="p_kt")        # rope'd K^T
            va_sb = persist.tile([128, KT, 256], fp16, tag="p_va")  # V natural + ones col (256B-aligned rows for the xbar transpose)
            at_sb = persist.tile([128, HPC, S], fp16, tag="p_at")   # A^T per head
            nc.gpsimd.memset(va_sb[:, :, 128:130], 1.0)

            # ---- phase 1: projections + rope, one seq-quarter at a time ----
            # unit order per quarter: K, V, Q0..Q3 (K needs only wk + first chunks)
            pending = []

            def finish_unit(kind, raw, q0, u):
                if kind == "v":
                    kt0 = q0 // 128
                    for j in range(4):
                        nc.scalar.dma_start_transpose(
                            va_sb[:, kt0 + j, 0:128],
                            raw[:, j * 128:(j + 1) * 128],
                        )
                else:
                    sw_ps = ps_acc.tile([128, QB], f32, tag="ps_acc", name="sw_ps")
                    nc.tensor.matmul(sw_ps[:], sw_sb[:], raw[:], start=True, stop=True)
                    t1 = t12.tile([128, QB], fp16, tag="t12")
                    nc.vector.tensor_mul(t1[:], raw[:], rc_sb[:, q0:q0 + QB])
                    t2 = t12.tile([128, QB], fp16, tag="t12")
                    nc.vector.tensor_mul(t2[:], sw_ps[:], rs_sb[:, q0:q0 + QB])
                    if kind == "q":
                        dest = qt_sb[:, u, q0:q0 + QB]
                    else:
                        dest = kt_sb[:, q0:q0 + QB]
                    nc.vector.tensor_add(dest, t1[:], t2[:])

            def prefetch_xt(qi, j):
                if qi >= NQB:
                    return
                lst = xt_tiles.setdefault(qi, [])
                if len(lst) > j:
                    return
                nq0 = qi * QB
                t = xtp.tile([128, 8, QB], fp16, tag="xt", name=f"xt_{qi}_{j}")
                nc.sync.dma_start(t[:], xt_r[:, 8 * j:8 * (j + 1), nq0:nq0 + QB])
                lst.append(t)

            def w_slice(kind, h, kt):
                if kind == "q":
                    return wq_c[kt // 8][:, kt % 8, h * HD:(h + 1) * HD]
                if kind == "k":
                    return wkv_c[kt // 8][:, kt % 8, 0:HD]
                return wkv_c[kt // 8][:, kt % 8, HD:2 * HD]

            GROUPS = [[("k", -1), ("v", -1), ("q", 0)], [("q", 1), ("q", 2), ("q", 3)]]
            for qi in range(NQB):
                q0 = qi * QB
                xt_c = xt_tiles[qi]
                for gi, grp in enumerate(GROUPS):
                    if gi == 0:
                        prefetch_xt(qi + 1, 0)
                        prefetch_xt(qi + 1, 1)
                    else:
                        prefetch_xt(qi + 1, 2)
                        prefetch_xt(qi + 1, 3)
                    pss = [
                        ps_acc.tile([128, QB], f32, tag="ps_acc", name=f"pj{gi}{gu}")
                        for gu in range(3)
                    ]
                    for j in range(4):
                        for gu, (kind, h) in enumerate(grp):
                            for kt in range(8 * j, 8 * j + 8):
                                nc.tensor.matmul(
                                    pss[gu][:],
                                    w_slice(kind, h, kt),
                                    xt_c[kt // 8][:, kt % 8, :],
                                    start=(kt == 0),
                                    stop=(kt == DKT - 1),
                                )
                        if pending:
                            finish_unit(*pending.pop(0))
                    for gu, (kind, h) in enumerate(grp):
                        raw = tmp.tile([128, QB], fp16, tag="tmp")
                        nc.scalar.copy(raw[:], pss[gu][:])
                        pending.append((kind, raw, q0, h))
            while pending:
                finish_unit(*pending.pop(0))

            nc.scalar.dma_start(wo_sb[:], wo_r[:])

            # ---- phase 2+3: attention pipelined across (qblock, head) ----
            def wo_block(qi):
                for sti in range(4):
                    st = qi * 4 + sti
                    o_sb = outp.tile([128, DIM], fp16, tag="outp")
                    for t in range(4):
                        wo_ps = ps_big.tile([128, 1024], f32, tag="ps_big", name="wo_ps")
                        for h in range(HPC):
                            for half in range(2):
                                nb = t * 2 + half
                                nc.tensor.matmul(
                                    wo_ps[:, half * 512:(half + 1) * 512],
                                    at_sb[:, h, st * 128:(st + 1) * 128],
                                    wo_sb[:, h, nb * 512:(nb + 1) * 512],
                                    start=(h == 0),
                                    stop=(h == HPC - 1),
                                )
                        nc.vector.tensor_copy(o_sb[:, t * 1024:(t + 1) * 1024], wo_ps[:])
                    nc.gpsimd.dma_start(out_r[st], o_sb[:])

            def divide_head(acc, q0, h):
                # normalize (on DVE only; keeps ACT exp stream and PE unblocked)
                for qs in range(4):
                    linv = small.tile([128, 1], f32, tag="small")
                    nc.vector.reciprocal(linv[:], acc[qs][:, 128:129])
                    a_sb = asbp.tile([128, 128], fp16, tag="asb")
                    nc.vector.tensor_scalar_mul(a_sb[:], acc[qs][:, 0:128], linv[:, 0:1])
                    nc.sync.dma_start_transpose(
                        at_sb[:, h, q0 + qs * 128:q0 + (qs + 1) * 128], a_sb[:]
                    )

            prev = None       # (p_tile, kt_pair, acc)
            for qi in range(NQB):
                q0 = qi * QB
                for h in range(HPC):
                    acc = [
                        ps_acc.tile([128, 132], f32, tag="ps_acc", name=f"acc{i}")
                        for i in range(4)
                    ]
                    for kp in range(NPAIR):
                        s_ps = ps_big.tile([128, 1024], f32, tag="ps_big", name="s_ps")
                        for j in range(2):
                            nc.tensor.matmul(
                                s_ps[:, j * QB:(j + 1) * QB],
                                kt_sb[:, (2 * kp + j) * 128:(2 * kp + j + 1) * 128],
                                qt_sb[:, h, q0:q0 + QB],
                                start=True,
                                stop=True,
                            )
                        p_t = ptp.tile([128, 1024], fp16, tag="pt")
                        nc.scalar.activation(
                            p_t[:], s_ps[:], AF.Exp, bias=ebias_sb[:, 0:1], scale=SCALE
                        )
                        if prev is not None:
                            pp, pkp, pacc = prev
                            for j in range(2):
                                kt = 2 * pkp + j
                                for qs in range(4):
                                    nc.tensor.matmul(
                                        pacc[qs][:, 0:129],
                                        pp[:, j * QB + qs * 128:j * QB + (qs + 1) * 128],
                                        va_sb[:, kt, 0:129],
                                        start=(kt == 0),
                                        stop=(kt == KT - 1),
                                    )
                            if pkp == NPAIR - 1:
                                # previous head fully accumulated: finish it
                                ph = h - 1
                                pq0 = q0
                                if ph < 0:
                                    ph, pq0 = HPC - 1, q0 - QB
                                divide_head(pacc, pq0, ph)
                        if h == 0 and kp == 2 and qi > 0:
                            wo_block(qi - 1)
                        prev = (p_t, kp, acc)
            # drain the pipeline: last head's PV, divide, final wo block
            pp, pkp, pacc = prev
            for j in range(2):
                kt = 2 * pkp + j
                for qs in range(4):
                    nc.tensor.matmul(
                        pacc[qs][:, 0:129],
                        pp[:, j * QB + qs * 128:j * QB + (qs + 1) * 128],
                        va_sb[:, kt, 0:129],
                        start=(kt == 0),
                        stop=(kt == KT - 1),
                    )
            divide_head(pacc, (NQB - 1) * QB, HPC - 1)
            wo_block(NQB - 1)

    nc.compile()
    return nc


def _get_nc():
    if "nc" not in _CACHE:
        _CACHE["nc"] = _build_nc()
    return _CACHE["nc"]


def _make_in_maps(x, freqs_cos, freqs_sin, wq, wk, wv, wo):
    x = np.asarray(x, dtype=np.float32)
    freqs_cos = np.asarray(freqs_cos, dtype=np.float32)
    freqs_sin = np.asarray(freqs_sin, dtype=np.float32)
    wq = np.asarray(wq, dtype=np.float32)
    wk = np.asarray(wk, dtype=np.float32)
    wv = np.asarray(wv, dtype=np.float32)
    wo = np.asarray(wo, dtype=np.float32)
    xt = np.ascontiguousarray(x.T).astype(np.float16)
    rc = np.repeat(freqs_cos.T, 2, axis=0).astype(np.float16)
    sgn = np.where(np.arange(HD) % 2 == 0, -1.0, 1.0)[:, None].astype(np.float32)
    rs = (np.repeat(freqs_sin.T, 2, axis=0) * sgn).astype(np.float16)
    sw = np.zeros((HD, HD), np.float16)
    idx = np.arange(HD)
    sw[idx, idx ^ 1] = 1.0
    in_maps = []
    for c in range(NCORES):
        in_maps.append({
            "xt": xt,
            "wq": np.ascontiguousarray(wq[:, c * 512:(c + 1) * 512]).astype(np.float16),
            "wkv": np.ascontiguousarray(np.concatenate(
                [wk[:, c * 128:(c + 1) * 128], wv[:, c * 128:(c + 1) * 128]],
                axis=1)).astype(np.float16),
            "wo": np.ascontiguousarray(wo[c * 512:(c + 1) * 512, :]).astype(np.float16),
            "ropec": rc,
            "ropes": rs,
            "pswap": sw,
        })
    return in_maps


def _run(inputs, trace=False):
    from concourse.bass_utils import run_bass_kernel_spmd

    nc = _get_nc()
    in_maps = _make_in_maps(**inputs)
    res = run_bass_kernel_spmd(nc, in_maps, core_ids=list(range(NCORES)), trace=trace)
    parts = [r["out"].astype(np.float32) for r in res.results]
    out = np.sum(np.stack(parts), axis=0)
    return out, res


def kernel(**inputs) -> np.ndarray:
    out, _ = _run(inputs, trace=False)
    return out


# revision 34
# speedup vs baseline: 1.0022x; 1.0022x over previous
"""Trainium2 Bass kernel: GQA attention block (nn_Attention_66142496358763).

Full module: x -> (wq,wk,wv) projections -> RoPE(q,k) -> softmax(q k^T/sqrt(d)) v
(GQA: 32 q heads, 8 kv heads) -> wo projection.

Sharding (tensor-parallel over heads, 8 cores):
  core c: q heads [4c, 4c+4), kv head c, wq/wk/wv column shards, wo row shard
  -> each core emits a partial [S, DIM] output; host sums the 8 partials.

All TensorE math in fp16 (full-rate on trn2), fp32 PSUM accumulation,
softmax exp in fp32 on ScalarE. Softmax is computed without the max
subtraction (scores are O(10) here; a -4 bias inside exp keeps the fp16
P-matrix in range) and the denominator comes for free from a ones-column
appended to V inside the PV matmul. A/V transposes ride the DMA xbar.
"""

import numpy as np

S = 2048
DIM = 4096
HD = 128
NCORES = 8
HPC = 4          # q heads per core
QB = 512         # q block (seq block) size
NQB = S // QB    # 4
DKT = DIM // 128  # 32 contraction tiles for projections
KT = S // 128    # 16 key tiles for attention
NPAIR = KT // 2  # 8 score/exp pairs per (head, qblock)
SCALE = float(HD) ** -0.5
EXP_BIAS = -4.0

_CACHE = {}


def _build_nc():
    import concourse.bass as bass
    import concourse.tile as tile
    from concourse import bacc, mybir

    fp16 = mybir.dt.float16
    f32 = mybir.dt.float32
    AF = mybir.ActivationFunctionType

    nc = bacc.Bacc("TRN2", target_bir_lowering=False, debug=False)

    xt_d = nc.dram_tensor("xt", [DIM, S], fp16, kind="ExternalInput").ap()
    wq_d = nc.dram_tensor("wq", [DIM, HPC * HD], fp16, kind="ExternalInput").ap()
    wkv_d = nc.dram_tensor("wkv", [DIM, 2 * HD], fp16, kind="ExternalInput").ap()
    wo_d = nc.dram_tensor("wo", [HPC * HD, DIM], fp16, kind="ExternalInput").ap()
    rc_d = nc.dram_tensor("ropec", [HD, S], fp16, kind="ExternalInput").ap()
    rs_d = nc.dram_tensor("ropes", [HD, S], fp16, kind="ExternalInput").ap()
    sw_d = nc.dram_tensor("pswap", [HD, HD], fp16, kind="ExternalInput").ap()
    out_d = nc.dram_tensor("out", [S, DIM], fp16, kind="ExternalOutput").ap()

    xt_r = xt_d.rearrange("(kt p) s -> p kt s", p=128)
    wq_r = wq_d.rearrange("(kt p) n -> p kt n", p=128)
    wkv_r = wkv_d.rearrange("(kt p) n -> p kt n", p=128)
    wo_r = wo_d.rearrange("(h p) n -> p h n", p=128)
    out_r = out_d.rearrange("(st p) n -> st p n", p=128)

    with tile.TileContext(nc) as tc:
        with (
            tc.tile_pool(name="const", bufs=1) as const,
            tc.tile_pool(name="xtp", bufs=5) as xtp,
            tc.tile_pool(name="persist", bufs=1) as persist,
            tc.tile_pool(name="tmp", bufs=5) as tmp,
            tc.tile_pool(name="t12", bufs=4) as t12,
            tc.tile_pool(name="pt", bufs=4) as ptp,
            tc.tile_pool(name="asb", bufs=6) as asbp,
            tc.tile_pool(name="small", bufs=8) as small,
            tc.tile_pool(name="outp", bufs=2) as outp,
            tc.tile_pool(name="ps_big", bufs=2, space="PSUM") as ps_big,
            tc.tile_pool(name="ps_acc", bufs=4, space="PSUM") as ps_acc,
        ):
            # ---- inputs split across the two HWDGE queues:
            # sync: wk + the xt chunk stream; scalar: rope consts, wv, wq, wo
            xt_tiles = {0: []}
            for j in range(4):
                t = xtp.tile([128, 8, QB], fp16, tag="xt", name=f"xt_0_{j}")
                nc.sync.dma_start(t[:], xt_r[:, 8 * j:8 * (j + 1), 0:QB])
                xt_tiles[0].append(t)
            wkv_c = []
            wq_c = []
            for j in range(4):
                w = const.tile([128, 8, 2 * HD], fp16, tag=f"c_wkv{j}", name=f"wkv_c{j}")
                nc.scalar.dma_start(w[:], wkv_r[:, 8 * j:8 * (j + 1), :])
                wkv_c.append(w)
                q = const.tile([128, 8, 512], fp16, tag=f"c_wq{j}", name=f"wq_c{j}")
                nc.scalar.dma_start(q[:], wq_r[:, 8 * j:8 * (j + 1), :])
                wq_c.append(q)
            rc_sb = const.tile([HD, S], fp16, tag="c_rc")
            nc.scalar.dma_start(rc_sb[:], rc_d[:])
            rs_sb = const.tile([HD, S], fp16, tag="c_rs")
            nc.scalar.dma_start(rs_sb[:], rs_d[:])
            sw_sb = const.tile([HD, HD], fp16, tag="c_sw")
            nc.scalar.dma_start(sw_sb[:], sw_d[:])
            ebias_sb = const.tile([128, 1], f32, tag="c_eb")
            nc.gpsimd.memset(ebias_sb[:], EXP_BIAS)
            wo_sb = const.tile([128, HPC, DIM], fp16, tag="c_wo")

            # PE warm-up: ~9us of dummy matmuls with no input deps, so the HAM
            # clock-gate opens while the first DMAs are still streaming in
            warm_sb = const.tile([128, 128], fp16, tag="c_warm")
            nc.gpsimd.memset(warm_sb[:], 0.0)
            warm_ps = ps_big.tile([128, 1024], f32, tag="ps_big", name="warm_ps")
            for _ in range(80):
                nc.tensor.matmul(
                    warm_ps[:, 0:128], warm_sb[:], warm_sb[:], start=True, stop=True
                )

            # persistent activations
            qt_sb = persist.tile([128, HPC, S], fp16, tag="p_qt")   # rope'd Q^T per head
            kt_sb = persist.tile([128, S], fp16, tag="p_kt")        # rope'd K^T
            va_sb = persist.tile([128, KT, 256], fp16, tag="p_va")  # V natural + ones col (256B-aligned rows for the xbar transpose)
            at_sb = persist.tile([128, HPC, S], fp16, tag="p_at")   # A^T per head
            nc.gpsimd.memset(va_sb[:, :, 128:130], 1.0)

            # ---- phase 1: projections + rope, one seq-quarter at a time ----
            # unit order per quarter: K, V, Q0..Q3 (K needs only wk + first chunks)
            pending = []

            def finish_unit(kind, raw, q0, u):
                if kind == "v":
                    kt0 = q0 // 128
                    for j in range(4):
                        nc.scalar.dma_start_transpose(
                            va_sb[:, kt0 + j, 0:128],
                            raw[:, j * 128:(j + 1) * 128],
                        )
                else:
                    sw_ps = ps_acc.tile([128, QB], f32, tag="ps_acc", name="sw_ps")
                    nc.tensor.matmul(sw_ps[:], sw_sb[:], raw[:], start=True, stop=True)
                    t1 = t12.tile([128, QB], fp16, tag="t12")
                    nc.vector.tensor_mul(t1[:], raw[:], rc_sb[:, q0:q0 + QB])
                    t2 = t12.tile([128, QB], fp16, tag="t12")
                    nc.vector.tensor_mul(t2[:], sw_ps[:], rs_sb[:, q0:q0 + QB])
                    if kind == "q":
                        dest = qt_sb[:, u, q0:q0 + QB]
                    else:
                        dest = kt_sb[:, q0:q0 + QB]
                    nc.vector.tensor_add(dest, t1[:], t2[:])

            def prefetch_xt(qi, j):
                if qi >= NQB:
                    return
                lst = xt_tiles.setdefault(qi, [])
                if len(lst) > j:
                    return
                nq0 = qi * QB
                t = xtp.tile([128, 8, QB], fp16, tag="xt", name=f"xt_{qi}_{j}")
                nc.sync.dma_start(t[:], xt_r[:, 8 * j:8 * (j + 1), nq0:nq0 + QB])
                lst.append(t)

            def w_slice(kind, h, kt):
                if kind == "q":
                    return wq_c[kt // 8][:, kt % 8, h * HD:(h + 1) * HD]
                if kind == "k":
                    return wkv_c[kt // 8][:, kt % 8, 0:HD]
                return wkv_c[kt // 8][:, kt % 8, HD:2 * HD]

            GROUPS = [[("k", -1), ("v", -1), ("q", 0)], [("q", 1), ("q", 2), ("q", 3)]]
            for qi in range(NQB):
                q0 = qi * QB
                xt_c = xt_tiles[qi]
                for gi, grp in enumerate(GROUPS):
                    if gi == 0:
                        prefetch_xt(qi + 1, 0)
                        prefetch_xt(qi + 1, 1)
                    else:
                        prefetch_xt(qi + 1, 2)
                        prefetch_xt(qi + 1, 3)
                    pss = [
                        ps_acc.tile([128, QB], f32, tag="ps_acc", name=f"pj{gi}{gu}")
                        for gu in range(3)
                    ]
                    for j in range(4):
                        for gu, (kind, h) in enumerate(grp):
                            for kt in range(8 * j, 8 * j + 8):
                                nc.tensor.matmul(
                                    pss[gu][:],
                                    w_slice(kind, h, kt),
                                    xt_c[kt // 8][:, kt % 8, :],
                                    start=(kt == 0),
                                    stop=(kt == DKT - 1),
                                )
                        if pending:
                            finish_unit(*pending.pop(0))
                    for gu, (kind, h) in enumerate(grp):
                        raw = tmp.tile([128, QB], fp16, tag="tmp")
                        nc.scalar.copy(raw[:], pss[gu][:])
                        pending.append((kind, raw, q0, h))
            while pending:
                finish_unit(*pending.pop(0))

            nc.scalar.dma_start(wo_sb[:], wo_r[:])

            # ---- phase 2+3: attention pipelined across (qblock, head) ----
            def wo_block(qi, split_last=False):
                for sti in range(4):
                    st = qi * 4 + sti
                    halves = split_last and sti == 3
                    if halves:
                        o_sbs = [
                            outp.tile([128, DIM // 2], fp16, tag="outp", name=f"oh{k}")
                            for k in range(2)
                        ]
                    else:
                        o_sb = outp.tile([128, DIM], fp16, tag="outp")
                    for t in range(4):
                        wo_ps = ps_big.tile([128, 1024], f32, tag="ps_big", name="wo_ps")
                        for h in range(HPC):
                            for half in range(2):
                                nb = t * 2 + half
                                nc.tensor.matmul(
                                    wo_ps[:, half * 512:(half + 1) * 512],
                                    at_sb[:, h, st * 128:(st + 1) * 128],
                                    wo_sb[:, h, nb * 512:(nb + 1) * 512],
                                    start=(h == 0),
                                    stop=(h == HPC - 1),
                                )
                        if halves:
                            dest = o_sbs[t // 2][:, (t % 2) * 1024:(t % 2 + 1) * 1024]
                        else:
                            dest = o_sb[:, t * 1024:(t + 1) * 1024]
                        nc.vector.tensor_copy(dest, wo_ps[:])
                        if halves and t == 1:
                            nc.gpsimd.dma_start(out_r[st][:, 0:2048], o_sbs[0][:])
                    if halves:
                        nc.gpsimd.dma_start(out_r[st][:, 2048:4096], o_sbs[1][:])
                    else:
                        nc.gpsimd.dma_start(out_r[st], o_sb[:])

            def divide_head(acc, q0, h):
                # normalize (on DVE only; keeps ACT exp stream and PE unblocked)
                for qs in range(4):
                    linv = small.tile([128, 1], f32, tag="small")
                    nc.vector.reciprocal(linv[:], acc[qs][:, 128:129])
                    a_sb = asbp.tile([128, 128], fp16, tag="asb")
                    nc.vector.tensor_scalar_mul(a_sb[:], acc[qs][:, 0:128], linv[:, 0:1])
                    nc.sync.dma_start_transpose(
                        at_sb[:, h, q0 + qs * 128:q0 + (qs + 1) * 128], a_sb[:]
                    )

            prev = None       # (p_tile, kt_pair, acc)
            for qi in range(NQB):
                q0 = qi * QB
                for h in range(HPC):
                    acc = [
                        ps_acc.tile([128, 132], f32, tag="ps_acc", name=f"acc{i}")
                        for i in range(4)
                    ]
                    for kp in range(NPAIR):
                        s_ps = ps_big.tile([128, 1024], f32, tag="ps_big", name="s_ps")
                        for j in range(2):
                            nc.tensor.matmul(
                                s_ps[:, j * QB:(j + 1) * QB],
                                kt_sb[:, (2 * kp + j) * 128:(2 * kp + j + 1) * 128],
                                qt_sb[:, h, q0:q0 + QB],
                                start=True,
                                stop=True,
                            )
                        p_t = ptp.tile([128, 1024], fp16, tag="pt")
                        nc.scalar.activation(
                            p_t[:], s_ps[:], AF.Exp, bias=ebias_sb[:, 0:1], scale=SCALE
                        )
                        if prev is not None:
                            pp, pkp, pacc = prev
                            for j in range(2):
                                kt = 2 * pkp + j
                                for qs in range(4):
                                    nc.tensor.matmul(
                                        pacc[qs][:, 0:129],
                                        pp[:, j * QB + qs * 128:j * QB + (qs + 1) * 128],
                                        va_sb[:, kt, 0:129],
                                        start=(kt == 0),
                                        stop=(kt == KT - 1),
                                    )
                            if pkp == NPAIR - 1:
                                # previous head fully accumulated: finish it
                                ph = h - 1
                                pq0 = q0
                                if ph < 0:
                                    ph, pq0 = HPC - 1, q0 - QB
                                divide_head(pacc, pq0, ph)
                        if h == 0 and kp == 2 and qi > 0:
                            wo_block(qi - 1)
                        prev = (p_t, kp, acc)
            # drain the pipeline: last head's PV, divide, final wo block
            pp, pkp, pacc = prev
            for j in range(2):
                kt = 2 * pkp + j
                for qs in range(4):
                    nc.tensor.matmul(
                        pacc[qs][:, 0:129],
                        pp[:, j * QB + qs * 128:j * QB + (qs + 1) * 128],
                        va_sb[:, kt, 0:129],
                        start=(kt == 0),
                        stop=(kt == KT - 1),
                    )
            divide_head(pacc, (NQB - 1) * QB, HPC - 1)
            wo_block(NQB - 1, split_last=True)

    nc.compile()
    return nc


def _get_nc():
    if "nc" not in _CACHE:
        _CACHE["nc"] = _build_nc()
    return _CACHE["nc"]


def _make_in_maps(x, freqs_cos, freqs_sin, wq, wk, wv, wo):
    x = np.asarray(x, dtype=np.float32)
    freqs_cos = np.asarray(freqs_cos, dtype=np.float32)
    freqs_sin = np.asarray(freqs_sin, dtype=np.float32)
    wq = np.asarray(wq, dtype=np.float32)
    wk = np.asarray(wk, dtype=np.float32)
    wv = np.asarray(wv, dtype=np.float32)
    wo = np.asarray(wo, dtype=np.float32)
    xt = np.ascontiguousarray(x.T).astype(np.float16)
    rc = np.repeat(freqs_cos.T, 2, axis=0).astype(np.float16)
    sgn = np.where(np.arange(HD) % 2 == 0, -1.0, 1.0)[:, None].astype(np.float32)
    rs = (np.repeat(freqs_sin.T, 2, axis=0) * sgn).astype(np.float16)
    sw = np.zeros((HD, HD), np.float16)
    idx = np.arange(HD)
    sw[idx, idx ^ 1] = 1.0
    in_maps = []
    for c in range(NCORES):
        in_maps.append({
            "xt": xt,
            "wq": np.ascontiguousarray(wq[:, c * 512:(c + 1) * 512]).astype(np.float16),
            "wkv": np.ascontiguousarray(np.concatenate(
                [wk[:, c * 128:(c + 1) * 128], wv[:, c * 128:(c + 1) * 128]],
                axis=1)).astype(np.float16),
            "wo": np.ascontiguousarray(wo[c * 512:(c + 1) * 512, :]).astype(np.float16),
            "ropec": rc,
            "ropes": rs,
            "pswap": sw,
        })
    return in_maps


def _run(inputs, trace=False):
    from concourse.bass_utils import run_bass_kernel_spmd

    nc = _get_nc()
    in_maps = _make_in_maps(**inputs)
    res = run_bass_kernel_spmd(nc, in_maps, core_ids=list(range(NCORES)), trace=trace)
    parts = [r["out"].astype(np.float32) for r in res.results]
    out = np.sum(np.stack(parts), axis=0)
    return out, res


def kernel(**inputs) -> np.ndarray:
    out, _ = _run(inputs, trace=False)
    return out
